# revision 1
# baseline (speedup 1.0000x reference)
"""GATv2 2-layer encoder on 8 TRN2 NeuronCores.

Strategy: destination-node sharding. Nodes are bin-packed into 392 tiles of
128 slots each (balancing in-edge counts), 49 tiles per core. All edges
(incl. self-loops) are grouped by the tile owning their destination; each
tile's edges are padded to BPT blocks of 128. Per edge-block the kernel
gathers xl[src], e[rel], xr[dst] rows (indirect DMA, summed in the DMA
datapath), applies leaky-relu + per-head att dot to get logits, exp (softmax
without max-subtraction — logits are O(1)), and scatter-adds the weighted
source features into the tile's PSUM accumulator with a one-hot matmul.
Segment softmax needs no cross-core traffic; the only collectives are
AllGathers of the per-core node-feature table shards between layers.
"""
import sys
import heapq

import numpy as np

sys.path.insert(0, "/opt/trn_rl_repo")

import ml_dtypes  # noqa: E402
import concourse.bass as bass  # noqa: E402
import concourse.tile as tile  # noqa: E402
from concourse import bacc, mybir  # noqa: E402
from concourse.bass_utils import run_bass_kernel_spmd  # noqa: E402
from concourse.masks import make_identity  # noqa: E402

N, E, R = 50000, 400000, 500
IN, HID, H, OUT = 128, 64, 4, 128
HC1, HC2 = H * HID, H * OUT  # 256, 512
W = 8            # cores
P = 128          # partitions / tile slots / edge-block size
NT = 49          # node tiles per core
TILES = W * NT   # 392
NSLOT = TILES * P  # 50176
SHARD = NT * P   # 6272 rows per core
RPAD = 512       # padded relation table rows (row R = zeros for self-loops)

F32 = mybir.dt.float32
BF16 = mybir.dt.bfloat16
I32 = mybir.dt.int32
BF = ml_dtypes.bfloat16


def _preprocess(edge_index):
    """Self-loops, balanced node->tile binning, per-core block index arrays."""
    src = np.asarray(edge_index[0], dtype=np.int64)
    rel = np.asarray(edge_index[1], dtype=np.int64)
    dst = np.asarray(edge_index[2], dtype=np.int64)
    loop = np.arange(N, dtype=np.int64)
    src_f = np.concatenate([src, loop])
    dst_f = np.concatenate([dst, loop])
    rel_f = np.concatenate([rel, np.full(N, R, dtype=np.int64)])

    deg = np.bincount(dst_f, minlength=N)

    # Greedy balanced binning: highest-degree node to lightest non-full tile.
    order = np.argsort(-deg, kind="stable")
    tile_of = np.empty(N, np.int64)
    slot_of = np.empty(N, np.int64)
    heap = [(0, t) for t in range(TILES)]
    heapq.heapify(heap)
    counts = np.zeros(TILES, np.int64)
    loads = np.zeros(TILES, np.int64)
    for n in order:
        while True:
            load, t = heapq.heappop(heap)
            if counts[t] < P:
                break
        tile_of[n] = t
        slot_of[n] = counts[t]
        counts[t] += 1
        loads[t] += deg[n]
        if counts[t] < P:
            heapq.heappush(heap, (loads[t], t))

    perm_pos = tile_of * P + slot_of  # node -> row in permuted table layout

    bpt = max(1, int(-(-loads.max() // P)))  # blocks per tile (uniform)
    nblk = NT * bpt
    cap = bpt * P

    # Edge slots per tile, padded to cap.
    et = tile_of[dst_f]
    eorder = np.argsort(et, kind="stable")
    et_s = et[eorder]
    starts = np.searchsorted(et_s, np.arange(TILES))
    ends = np.searchsorted(et_s, np.arange(TILES), side="right")

    src_a = np.zeros((TILES, cap), np.int64)
    rel_a = np.full((TILES, cap), R, np.int64)
    dst_a = np.zeros((TILES, cap), np.int64)
    seg_a = np.full((TILES, cap), 999, np.int64)  # 999 => zero Q row (pad)
    for t in range(TILES):
        idx = eorder[starts[t]:ends[t]]
        k = idx.shape[0]
        src_a[t, :k] = src_f[idx]
        rel_a[t, :k] = rel_f[idx]
        dst_a[t, :k] = dst_f[idx]
        seg_a[t, :k] = slot_of[dst_f[idx]]

    # Per-core arrays. gidx layout: per block b cols [3b, 3b+1, 3b+2] =
    # (src-row, rel-row, dst-row); cols [3*nblk + t] = tile t's node row.
    gidx1 = np.zeros((W, P, nblk * 3 + NT), np.int32)
    gidx2 = np.zeros((W, P, nblk * 3 + NT), np.int32)
    qh = np.zeros((W, nblk * P, P), BF)
    ph = np.zeros((W, nblk * P, P), BF)
    node_of_slot = np.full(NSLOT, N, np.int64)  # pad slots -> zero x row
    node_of_slot[perm_pos] = np.arange(N)
    eye = np.eye(P, dtype=BF)
    zrow = np.zeros(P, BF)
    for c in range(W):
        for t in range(NT):
            g = c * NT + t
            s3 = src_a[g].reshape(bpt, P)
            r3 = rel_a[g].reshape(bpt, P)
            d3 = dst_a[g].reshape(bpt, P)
            sg3 = seg_a[g].reshape(bpt, P)
            for j in range(bpt):
                b = t * bpt + j
                gidx1[c, :, 3 * b + 0] = s3[j]
                gidx1[c, :, 3 * b + 1] = r3[j]
                gidx1[c, :, 3 * b + 2] = d3[j]
                rows = qh[c, b * P:(b + 1) * P]
                valid = sg3[j] < P
                rows[valid] = eye[sg3[j][valid]]
                rows[~valid] = zrow
                ph[c, b * P:(b + 1) * P] = rows.T
            gidx1[c, :, 3 * nblk + t] = node_of_slot[g * P:(g + 1) * P]
            gidx2[c, :, 3 * nblk + t] = np.arange(g * P, (g + 1) * P)
        # vectorized gidx2 block fill
        g0 = c * NT
        s_all = src_a[g0:g0 + NT].reshape(NT * bpt, P)
        r_all = rel_a[g0:g0 + NT].reshape(NT * bpt, P)
        d_all = dst_a[g0:g0 + NT].reshape(NT * bpt, P)
        gidx2[c, :, 0:3 * nblk:3] = perm_pos[s_all].T
        gidx2[c, :, 1:3 * nblk:3] = r_all.T
        gidx2[c, :, 2:3 * nblk:3] = perm_pos[d_all].T

    return dict(
        bpt=bpt, nblk=nblk, perm_pos=perm_pos, node_of_slot=node_of_slot,
        gidx1=gidx1, gidx2=gidx2, qh=qh, ph=ph,
    )


def _build(bpt, reps=1):
    nblk = NT * bpt
    nc = bacc.Bacc("TRN2", target_bir_lowering=False, debug=False, num_devices=W)

    # ---- per-core inputs
    x_shard = nc.declare_dram_parameter("x_shard", [SHARD, IN], F32, isOutput=False)
    gidx1 = nc.declare_dram_parameter("gidx1", [P, nblk * 3 + NT], I32, isOutput=False)
    gidx2 = nc.declare_dram_parameter("gidx2", [P, nblk * 3 + NT], I32, isOutput=False)
    qp = nc.declare_dram_parameter("qp", [nblk * P, 2 * P], BF16, isOutput=False)
    # ---- replicated inputs
    rel_pad = nc.declare_dram_parameter("rel_pad", [RPAD, IN], F32, isOutput=False)
    wl1 = nc.declare_dram_parameter("wl1", [IN, HC1], F32, isOutput=False)
    wr1 = nc.declare_dram_parameter("wr1", [IN, HC1], F32, isOutput=False)
    we1 = nc.declare_dram_parameter("we1", [IN, HC1], F32, isOutput=False)
    att1f = nc.declare_dram_parameter("att1f", [1, HC1], F32, isOutput=False)
    eb1 = nc.declare_dram_parameter("eb1", [1, HC1], F32, isOutput=False)
    ob1 = nc.declare_dram_parameter("ob1", [1, HC1], F32, isOutput=False)
    wl2 = nc.declare_dram_parameter("wl2", [HC1, HC2], F32, isOutput=False)
    wr2 = nc.declare_dram_parameter("wr2", [HC1, HC2], F32, isOutput=False)
    we2 = nc.declare_dram_parameter("we2", [IN, HC2], F32, isOutput=False)
    att2f = nc.declare_dram_parameter("att2f", [1, HC2], F32, isOutput=False)
    eb2 = nc.declare_dram_parameter("eb2", [1, HC2], F32, isOutput=False)
    ob2 = nc.declare_dram_parameter("ob2", [1, OUT], F32, isOutput=False)
    out_p = nc.declare_dram_parameter("out", [SHARD, OUT], F32, isOutput=True)

    # ---- internal DRAM
    e1t = nc.dram_tensor("e1t", [RPAD, HC1], BF16)
    e2t = nc.dram_tensor("e2t", [RPAD, HC2], BF16)
    xl_shard = nc.dram_tensor("xl_shard", [SHARD, HC1], BF16)
    xr_shard = nc.dram_tensor("xr_shard", [SHARD, HC1], BF16)
    xl1_full = nc.dram_tensor("xl1_full", [NSLOT, HC1], BF16, addr_space="Shared")
    xr1_full = nc.dram_tensor("xr1_full", [NSLOT, HC1], BF16, addr_space="Shared")
    h_shard = nc.dram_tensor("h_shard", [SHARD, HC1], BF16)
    xl2_shard = nc.dram_tensor("xl2_shard", [SHARD, HC2], BF16)
    xr2_shard = nc.dram_tensor("xr2_shard", [SHARD, HC2], BF16)
    xl2_full = nc.dram_tensor("xl2_full", [NSLOT, HC2], BF16, addr_space="Shared")
    xr2_full = nc.dram_tensor("xr2_full", [NSLOT, HC2], BF16, addr_space="Shared")

    RG = [list(range(W))]
    IOA = bass.IndirectOffsetOnAxis

    with tile.TileContext(nc) as tc:
        with (
            tc.tile_pool(name="const", bufs=1) as cp,
            tc.tile_pool(name="work", bufs=4) as wp,
            tc.tile_pool(name="ps1", bufs=1, space="PSUM") as ps1,
            tc.tile_pool(name="psacc", bufs=2, space="PSUM") as pa,
        ):
            for _rep in range(reps):
                # ================= consts =================
                ident = cp.tile([P, P], BF16)
                make_identity(nc, ident[:])
                wl1b = cp.tile([IN, HC1], BF16, tag="wl1b")
                nc.gpsimd.dma_start(out=wl1b[:], in_=wl1[:])
                wr1b = cp.tile([IN, HC1], BF16, tag="wr1b")
                nc.gpsimd.dma_start(out=wr1b[:], in_=wr1[:])
                we1b = cp.tile([IN, HC1], BF16, tag="we1b")
                nc.gpsimd.dma_start(out=we1b[:], in_=we1[:])
                we2b = cp.tile([IN, HC2], BF16, tag="we2b")
                nc.gpsimd.dma_start(out=we2b[:], in_=we2[:])
                wl2b = []
                wr2b = []
                for k in range(2):
                    wl2bk = cp.tile([P, HC2], BF16, tag=f"wl2b{k}")
                    nc.gpsimd.dma_start(out=wl2bk[:], in_=wl2[k * P:(k + 1) * P, :])
                    wl2b.append(wl2bk)
                    wr2bk = cp.tile([P, HC2], BF16, tag=f"wr2b{k}")
                    nc.gpsimd.dma_start(out=wr2bk[:], in_=wr2[k * P:(k + 1) * P, :])
                    wr2b.append(wr2bk)
                attB1 = cp.tile([P, HC1], BF16, tag="attB1")
                nc.gpsimd.dma_start(out=attB1[:], in_=att1f[:].to_broadcast([P, HC1]))
                attB2 = cp.tile([P, HC2], BF16, tag="attB2")
                nc.gpsimd.dma_start(out=attB2[:], in_=att2f[:].to_broadcast([P, HC2]))
                eb1B = cp.tile([P, HC1], F32, tag="eb1B")
                nc.sync.dma_start(out=eb1B[:], in_=eb1[:].to_broadcast([P, HC1]))
                ob1B = cp.tile([P, HC1], BF16, tag="ob1B")
                nc.gpsimd.dma_start(out=ob1B[:], in_=ob1[:].to_broadcast([P, HC1]))
                eb2B = cp.tile([P, HC2], F32, tag="eb2B")
                nc.sync.dma_start(out=eb2B[:], in_=eb2[:].to_broadcast([P, HC2]))
                ob2B = cp.tile([P, OUT], F32, tag="ob2B")
                nc.sync.dma_start(out=ob2B[:], in_=ob2[:].to_broadcast([P, OUT]))
                gidx1_t = cp.tile([P, nblk * 3 + NT], I32, tag="gidx1_t")
                nc.sync.dma_start(out=gidx1_t[:], in_=gidx1[:])
                gidx2_t = cp.tile([P, nblk * 3 + NT], I32, tag="gidx2_t")
                nc.sync.dma_start(out=gidx2_t[:], in_=gidx2[:])

                # ================= e-tables =================
                for k in range(RPAD // P):
                    rk = wp.tile([P, IN], BF16, tag="rk")
                    nc.gpsimd.dma_start(out=rk[:], in_=rel_pad[k * P:(k + 1) * P, :])
                    tp = ps1.tile([P, P], BF16, tag="tp")
                    nc.tensor.transpose(tp[:], rk[:], ident[:])
                    rT = wp.tile([P, IN], BF16, tag="rT")
                    nc.vector.tensor_copy(rT[:], tp[:])
                    psE1 = ps1.tile([P, HC2], F32, tag="psb")
                    nc.tensor.matmul(psE1[:, 0:HC1], lhsT=rT[:], rhs=we1b[:],
                                     start=True, stop=True)
                    e1sb = wp.tile([P, HC1], BF16, tag="e1sb")
                    nc.vector.tensor_tensor(out=e1sb[:], in0=psE1[:, 0:HC1], in1=eb1B[:],
                                            op=mybir.AluOpType.add)
                    nc.sync.dma_start(out=e1t[k * P:(k + 1) * P, :], in_=e1sb[:])
                    psE2 = ps1.tile([P, HC2], F32, tag="psb")
                    nc.tensor.matmul(psE2[:], lhsT=rT[:], rhs=we2b[:], start=True, stop=True)
                    e2sb = wp.tile([P, HC2], BF16, tag="e2sb")
                    nc.vector.tensor_tensor(out=e2sb[:], in0=psE2[:], in1=eb2B[:],
                                            op=mybir.AluOpType.add)
                    nc.sync.dma_start(out=e2t[k * P:(k + 1) * P, :], in_=e2sb[:])

                # ================= xl1/xr1 shard build =================
                for t in range(NT):
                    xt = wp.tile([P, IN], BF16, tag="xt")
                    nc.gpsimd.dma_start(out=xt[:], in_=x_shard[t * P:(t + 1) * P, :])
                    tp2 = ps1.tile([P, P], BF16, tag="tp")
                    nc.tensor.transpose(tp2[:], xt[:], ident[:])
                    xT = wp.tile([P, IN], BF16, tag="xT")
                    nc.vector.tensor_copy(xT[:], tp2[:])
                    psC = ps1.tile([P, HC2], F32, tag="psb")
                    nc.tensor.matmul(psC[:, 0:HC1], lhsT=xT[:], rhs=wl1b[:],
                                     start=True, stop=True)
                    nc.tensor.matmul(psC[:, HC1:HC2], lhsT=xT[:], rhs=wr1b[:],
                                     start=True, stop=True)
                    xlsb = wp.tile([P, HC1], BF16, tag="xlsb")
                    nc.scalar.activation(xlsb[:], psC[:, 0:HC1],
                                         mybir.ActivationFunctionType.Copy)
                    nc.sync.dma_start(out=xl_shard[t * P:(t + 1) * P, :], in_=xlsb[:])
                    xrsb = wp.tile([P, HC1], BF16, tag="xrsb")
                    nc.scalar.activation(xrsb[:], psC[:, HC1:HC2],
                                         mybir.ActivationFunctionType.Copy)
                    nc.sync.dma_start(out=xr_shard[t * P:(t + 1) * P, :], in_=xrsb[:])

                nc.gpsimd.collective_compute(
                    "AllGather", mybir.AluOpType.bypass,
                    ins=[xl_shard[:]], outs=[xl1_full[:]], replica_groups=RG)
                nc.gpsimd.collective_compute(
                    "AllGather", mybir.AluOpType.bypass,
                    ins=[xr_shard[:]], outs=[xr1_full[:]], replica_groups=RG)

                # ================= layer-1 edges =================
                for t in range(NT):
                    acc1 = pa.tile([P, HC1 + 4], F32, tag="accF")
                    XRTg = wp.tile([P, HC1], BF16, tag="XRTg")
                    nc.gpsimd.indirect_dma_start(
                        out=XRTg[:], out_offset=None, in_=xr1_full[:],
                        in_offset=IOA(
                            ap=gidx1_t[:, 3 * nblk + t:3 * nblk + t + 1], axis=0))
                    XRT = wp.tile([P, HC1], BF16, tag="XRT")
                    nc.vector.tensor_copy(XRT[:], XRTg[:])
                    for j in range(bpt):
                        b = t * bpt + j
                        QP = wp.tile([P, 2 * P], BF16, tag="QP")
                        nc.sync.dma_start(out=QP[:], in_=qp[b * P:(b + 1) * P, :])
                        Qb = QP[:, 0:P]
                        Pb = QP[:, P:2 * P]
                        Gl = wp.tile([P, HC1], BF16, tag="Gl")
                        nc.gpsimd.indirect_dma_start(
                            out=Gl[:], out_offset=None, in_=xl1_full[:],
                            in_offset=IOA(ap=gidx1_t[:, 3 * b:3 * b + 1], axis=0))
                        M = wp.tile([P, HC1], BF16, tag="M")
                        nc.gpsimd.indirect_dma_start(
                            out=M[:], out_offset=None, in_=e1t[:],
                            in_offset=IOA(ap=gidx1_t[:, 3 * b + 1:3 * b + 2], axis=0))
                        psX = pa.tile([P, HC1], F32, tag="psX")
                        nc.tensor.matmul(psX[:], lhsT=Pb, rhs=XRT[:],
                                         start=True, stop=False)
                        nc.tensor.matmul(psX[:], lhsT=ident[:], rhs=Gl[:],
                                         start=False, stop=True)
                        Xsb = wp.tile([P, HC1], BF16, tag="Xsb")
                        nc.scalar.activation(Xsb[:], psX[:],
                                             mybir.ActivationFunctionType.Copy)
                        Mf = wp.tile([P, HC1], BF16, tag="Mf")
                        nc.vector.tensor_tensor(out=Mf[:], in0=Xsb[:], in1=M[:],
                                                op=mybir.AluOpType.add)
                        Mr = wp.tile([P, HC1], BF16, tag="Mr")
                        nc.scalar.activation(Mr[:], Mf[:],
                                             mybir.ActivationFunctionType.Prelu, alpha=0.2)
                        T = wp.tile([P, HC1], BF16, tag="T")
                        nc.vector.tensor_tensor(out=T[:], in0=Mr[:], in1=attB1[:],
                                                op=mybir.AluOpType.mult)
                        logit = wp.tile([P, H], F32, tag="logit")
                        nc.vector.tensor_reduce(
                            out=logit[:], in_=T[:].rearrange("p (h c) -> p h c", h=H),
                            axis=mybir.AxisListType.X, op=mybir.AluOpType.add)
                        wf = wp.tile([P, H], F32, tag="wf")
                        nc.scalar.activation(wf[:], logit[:],
                                             mybir.ActivationFunctionType.Exp)
                        Rt = wp.tile([P, HC1 + 4], BF16, tag="Rt")
                        nc.scalar.activation(Rt[:, HC1:HC1 + 4], logit[:],
                                             mybir.ActivationFunctionType.Exp)
                        for hh in range(H):
                            nc.scalar.activation(
                                Rt[:, hh * HID:(hh + 1) * HID], Gl[:, hh * HID:(hh + 1) * HID],
                                mybir.ActivationFunctionType.Copy,
                                scale=wf[:, hh:hh + 1])
                        nc.tensor.matmul(acc1[:], lhsT=Qb, rhs=Rt[:],
                                         start=(j == 0), stop=(j == bpt - 1))
                    # epilogue: h = acc/denom + bias
                    dn1 = wp.tile([P, H], F32, tag="dn1")
                    nc.vector.tensor_scalar_add(dn1[:], acc1[:, HC1:HC1 + 4], 1e-20)
                    rec = wp.tile([P, H], F32, tag="rec")
                    nc.vector.reciprocal(rec[:], dn1[:])
                    htmp = wp.tile([P, HC1], BF16, tag="htmp")
                    for hh in range(H):
                        nc.scalar.activation(
                            htmp[:, hh * HID:(hh + 1) * HID],
                            acc1[:, hh * HID:(hh + 1) * HID],
                            mybir.ActivationFunctionType.Copy, scale=rec[:, hh:hh + 1])
                    hsb = wp.tile([P, HC1], BF16, tag="hsb")
                    nc.vector.tensor_tensor(out=hsb[:], in0=htmp[:], in1=ob1B[:],
                                            op=mybir.AluOpType.add)
                    nc.sync.dma_start(out=h_shard[t * P:(t + 1) * P, :], in_=hsb[:])

                # ================= xl2/xr2 build =================
                for t in range(NT):
                    ht = wp.tile([P, HC1], BF16, tag="ht")
                    nc.sync.dma_start(out=ht[:], in_=h_shard[t * P:(t + 1) * P, :])
                    hT = []
                    for k in range(2):
                        tp3 = ps1.tile([P, P], BF16, tag="tp")
                        nc.tensor.transpose(tp3[:], ht[:, k * P:(k + 1) * P], ident[:])
                        hTk = wp.tile([P, P], BF16, tag=f"hT{k}")
                        nc.vector.tensor_copy(hTk[:], tp3[:])
                        hT.append(hTk)
                    ps2l = ps1.tile([P, HC2], F32, tag="psb")
                    for k in range(2):
                        nc.tensor.matmul(ps2l[:], lhsT=hT[k][:], rhs=wl2b[k][:],
                                         start=(k == 0), stop=(k == 1))
                    xl2sb = wp.tile([P, HC2], BF16, tag="xl2sb")
                    nc.scalar.activation(xl2sb[:], ps2l[:], mybir.ActivationFunctionType.Copy)
                    nc.sync.dma_start(out=xl2_shard[t * P:(t + 1) * P, :], in_=xl2sb[:])
                    ps2r = ps1.tile([P, HC2], F32, tag="psb")
                    for k in range(2):
                        nc.tensor.matmul(ps2r[:], lhsT=hT[k][:], rhs=wr2b[k][:],
                                         start=(k == 0), stop=(k == 1))
                    xr2sb = wp.tile([P, HC2], BF16, tag="xr2sb")
                    nc.scalar.activation(xr2sb[:], ps2r[:], mybir.ActivationFunctionType.Copy)
                    nc.sync.dma_start(out=xr2_shard[t * P:(t + 1) * P, :], in_=xr2sb[:])

                nc.gpsimd.collective_compute(
                    "AllGather", mybir.AluOpType.bypass,
                    ins=[xl2_shard[:]], outs=[xl2_full[:]], replica_groups=RG)
                nc.gpsimd.collective_compute(
                    "AllGather", mybir.AluOpType.bypass,
                    ins=[xr2_shard[:]], outs=[xr2_full[:]], replica_groups=RG)

                # ================= layer-2 edges =================
                for t in range(NT):
                    acc2 = pa.tile([P, HC2], F32, tag="accF")
                    accd = pa.tile([P, 4], F32, tag="accD")
                    XRT2g = wp.tile([P, HC2], BF16, tag="XRT2g")
                    nc.gpsimd.indirect_dma_start(
                        out=XRT2g[:], out_offset=None, in_=xr2_full[:],
                        in_offset=IOA(
                            ap=gidx2_t[:, 3 * nblk + t:3 * nblk + t + 1], axis=0))
                    XRT2 = wp.tile([P, HC2], BF16, tag="XRT2")
                    nc.vector.tensor_copy(XRT2[:], XRT2g[:])
                    for j in range(bpt):
                        b = t * bpt + j
                        QP = wp.tile([P, 2 * P], BF16, tag="QP")
                        nc.sync.dma_start(out=QP[:], in_=qp[b * P:(b + 1) * P, :])
                        Qb = QP[:, 0:P]
                        Pb = QP[:, P:2 * P]
                        Gl2 = wp.tile([P, HC2], BF16, tag="Gl2")
                        nc.gpsimd.indirect_dma_start(
                            out=Gl2[:], out_offset=None, in_=xl2_full[:],
                            in_offset=IOA(ap=gidx2_t[:, 3 * b:3 * b + 1], axis=0))
                        M2 = wp.tile([P, HC2], BF16, tag="M2")
                        nc.gpsimd.indirect_dma_start(
                            out=M2[:], out_offset=None, in_=e2t[:],
                            in_offset=IOA(ap=gidx2_t[:, 3 * b + 1:3 * b + 2], axis=0))
                        psX2 = pa.tile([P, HC2], F32, tag="psX")
                        nc.tensor.matmul(psX2[:], lhsT=Pb, rhs=XRT2[:],
                                         start=True, stop=False)
                        nc.tensor.matmul(psX2[:], lhsT=ident[:], rhs=Gl2[:],
                                         start=False, stop=True)
                        Xsb2 = wp.tile([P, HC2], BF16, tag="Xsb2")
                        nc.scalar.activation(Xsb2[:], psX2[:],
                                             mybir.ActivationFunctionType.Copy)
                        Mf2 = wp.tile([P, HC2], BF16, tag="Mf2")
                        nc.vector.tensor_tensor(out=Mf2[:], in0=Xsb2[:], in1=M2[:],
                                                op=mybir.AluOpType.add)
                        Mr2 = wp.tile([P, HC2], BF16, tag="Mr2")
                        nc.scalar.activation(Mr2[:], Mf2[:],
                                             mybir.ActivationFunctionType.Prelu, alpha=0.2)
                        T2 = wp.tile([P, HC2], BF16, tag="T2")
                        nc.vector.tensor_tensor(out=T2[:], in0=Mr2[:], in1=attB2[:],
                                                op=mybir.AluOpType.mult)
                        logit2 = wp.tile([P, H], F32, tag="logit2")
                        nc.vector.tensor_reduce(
                            out=logit2[:], in_=T2[:].rearrange("p (h c) -> p h c", h=H),
                            axis=mybir.AxisListType.X, op=mybir.AluOpType.add)
                        wf2 = wp.tile([P, H], F32, tag="wf2")
                        nc.scalar.activation(wf2[:], logit2[:],
                                             mybir.ActivationFunctionType.Exp)
                        R2 = wp.tile([P, HC2 + 4], BF16, tag="R2")
                        nc.scalar.activation(R2[:, HC2:HC2 + 4], logit2[:],
                                             mybir.ActivationFunctionType.Exp)
                        for hh in range(H):
                            nc.scalar.activation(
                                R2[:, hh * OUT:(hh + 1) * OUT], Gl2[:, hh * OUT:(hh + 1) * OUT],
                                mybir.ActivationFunctionType.Copy,
                                scale=wf2[:, hh:hh + 1])
                        nc.tensor.matmul(acc2[:], lhsT=Qb, rhs=R2[:, 0:HC2],
                                         start=(j == 0), stop=(j == bpt - 1))
                        nc.tensor.matmul(accd[:], lhsT=Qb, rhs=R2[:, HC2:HC2 + 4],
                                         start=(j == 0), stop=(j == bpt - 1))
                    # epilogue: out = mean_h(acc_h/denom_h) + bias
                    dn2 = wp.tile([P, H], F32, tag="dn2")
                    nc.vector.tensor_scalar_add(dn2[:], accd[:], 1e-20)
                    rec2 = wp.tile([P, H], F32, tag="rec2")
                    nc.vector.reciprocal(rec2[:], dn2[:])
                    rec4 = wp.tile([P, H], F32, tag="rec4")
                    nc.vector.tensor_scalar_mul(rec4[:], rec2[:], 0.25)
                    hsum = []
                    for hh in range(H):
                        ho = wp.tile([P, OUT], F32, tag=f"ho{hh}")
                        nc.scalar.activation(
                            ho[:], acc2[:, hh * OUT:(hh + 1) * OUT],
                            mybir.ActivationFunctionType.Copy, scale=rec4[:, hh:hh + 1])
                        hsum.append(ho)
                    s01 = wp.tile([P, OUT], F32, tag="s01")
                    nc.vector.tensor_tensor(out=s01[:], in0=hsum[0][:], in1=hsum[1][:],
                                            op=mybir.AluOpType.add)
                    s23 = wp.tile([P, OUT], F32, tag="s23")
                    nc.vector.tensor_tensor(out=s23[:], in0=hsum[2][:], in1=hsum[3][:],
                                            op=mybir.AluOpType.add)
                    s0123 = wp.tile([P, OUT], F32, tag="s0123")
                    nc.vector.tensor_tensor(out=s0123[:], in0=s01[:], in1=s23[:],
                                            op=mybir.AluOpType.add)
                    osb = wp.tile([P, OUT], F32, tag="osb")
                    nc.vector.tensor_tensor(out=osb[:], in0=s0123[:], in1=ob2B[:],
                                            op=mybir.AluOpType.add)
                    nc.sync.dma_start(out=out_p[t * P:(t + 1) * P, :], in_=osb[:])

    nc.compile()
    return nc


def _make_in_maps(inp, pre):
    f32 = np.float32
    x_pad = np.zeros((NSLOT, IN), f32)
    x_pad[:N] = np.asarray(inp["x"], f32)
    rel_pad = np.zeros((RPAD, IN), f32)
    rel_pad[:R] = np.asarray(inp["relations"], f32)
    a = lambda k: np.asarray(inp[k], f32)
    rep = dict(
        rel_pad=rel_pad,
        wl1=a("Wl1"), wr1=a("Wr1"), we1=a("We1"),
        att1f=a("att1").reshape(1, HC1),
        eb1=(a("bl1") + a("br1")).reshape(1, HC1),
        ob1=(a("bl1") + a("bias1")).reshape(1, HC1),
        wl2=a("Wl2"), wr2=a("Wr2"), we2=a("We2"),
        att2f=a("att2").reshape(1, HC2),
        eb2=(a("bl2") + a("br2")).reshape(1, HC2),
        ob2=(a("bl2").reshape(H, OUT).mean(axis=0) + a("bias2")).reshape(1, OUT),
    )
    in_maps = []
    for c in range(W):
        m = dict(rep)
        m["x_shard"] = np.ascontiguousarray(x_pad[c * SHARD:(c + 1) * SHARD])
        m["gidx1"] = np.ascontiguousarray(pre["gidx1"][c])
        m["gidx2"] = np.ascontiguousarray(pre["gidx2"][c])
        m["qp"] = np.ascontiguousarray(
            np.concatenate([pre["qh"][c], pre["ph"][c]], axis=1))
        in_maps.append(m)
    return in_maps


_CACHE = {}


def kernel(x, edge_index, relations,
           Wl1, bl1, Wr1, br1, We1, att1, bias1,
           Wl2, bl2, Wr2, br2, We2, att2, bias2, **_unused):
    x = np.asarray(x, np.float32)
    edge_index = np.asarray(edge_index)
    relations = np.asarray(relations, np.float32)

    pre = _preprocess(edge_index)
    bpt = pre["bpt"]

    if bpt not in _CACHE:
        _CACHE[bpt] = _build(bpt)
    nc = _CACHE[bpt]

    in_maps = _make_in_maps(
        dict(x=x, relations=relations, Wl1=Wl1, bl1=bl1, Wr1=Wr1, br1=br1,
             We1=We1, att1=att1, bias1=bias1, Wl2=Wl2, bl2=bl2, Wr2=Wr2,
             br2=br2, We2=We2, att2=att2, bias2=bias2), pre)

    import os
    trace = os.environ.get("GAT_TRACE", "0") == "1"
    res = run_bass_kernel_spmd(nc, in_maps, list(range(W)), trace=trace)
    global LAST_EXEC_NS, LAST_RES
    LAST_EXEC_NS = res.exec_time_ns
    LAST_RES = res
    cat = np.concatenate([res.results[c]["out"] for c in range(W)], axis=0)
    return np.ascontiguousarray(cat[pre["perm_pos"]])


if __name__ == "__main__":
    pass



# revision 12
# speedup vs baseline: 1.0161x; 1.0161x over previous
"""GATv2 2-layer encoder on 8 TRN2 NeuronCores — v2.

Destination-node sharding in a permuted "slot" space: nodes are bin-packed
into 392 tiles of 128 slots (balancing in-edge counts), 49 tiles per core.
All full node tables (x, xl1, xl2) use a chunk-major row layout so the xl2
AllGather can be split into chunks and overlapped with the layer-1 edge
loop. Per tile, all bpt*128 edge rows are fetched with single batched
indirect DMAs (multi-column offset APs); e-embedding rows are summed onto
the xl rows in the DMA datapath (compute_op=add). One-hot scatter/broadcast
matrices are generated on-chip (iota + is_equal + PE transpose). Layer 1
needs no collective: each core computes the full xl1 table locally.
"""
import sys
import heapq

import numpy as np

sys.path.insert(0, "/opt/trn_rl_repo")

import ml_dtypes  # noqa: E402
import concourse.bass as bass  # noqa: E402
import concourse.tile as tile  # noqa: E402
from concourse import bacc, mybir  # noqa: E402
from concourse.bass_utils import run_bass_kernel_spmd  # noqa: E402
from concourse.masks import make_identity  # noqa: E402

N, E, R = 50000, 400000, 500
IN, HID, H, OUT = 128, 64, 4, 128
HC1, HC2 = H * HID, H * OUT  # 256, 512
W = 8            # cores
P = 128          # partitions / tile slots / edge-block size
NT = 49          # node tiles per core
TILES = W * NT   # 392
NSLOT = TILES * P  # 50176
SHARD = NT * P   # 6272 rows per core
RPAD = 512       # padded relation table rows (rows R.. are zero)

F32 = mybir.dt.float32
BF16 = mybir.dt.bfloat16
I32 = mybir.dt.int32
BF = ml_dtypes.bfloat16

# xl2 AllGather chunking (tiles per chunk); sum must be NT
CHUNK_TILES = [49]
CHUNK_T0 = np.cumsum([0] + CHUNK_TILES[:-1]).tolist()
CHUNK_BASE = np.cumsum(
    [0] + [W * ct * P for ct in CHUNK_TILES[:-1]]).tolist()
NCHUNK = len(CHUNK_TILES)


def _chunk_of_tile(t):
    for k in range(NCHUNK):
        if CHUNK_T0[k] <= t < CHUNK_T0[k] + CHUNK_TILES[k]:
            return k
    raise AssertionError


def _row0_of_tile(g):
    """First chunk-major DRAM row of global tile g in a full node table."""
    c, t = g // NT, g % NT
    k = _chunk_of_tile(t)
    return CHUNK_BASE[k] + c * CHUNK_TILES[k] * P + (t - CHUNK_T0[k]) * P


_LUT = None


def _chunkrow_lut():
    """slot (tile-major) -> chunk-major DRAM row."""
    global _LUT
    if _LUT is None:
        lut = np.empty(NSLOT, np.int64)
        for g in range(TILES):
            lut[g * P:(g + 1) * P] = _row0_of_tile(g) + np.arange(P)
        _LUT = lut
    return _LUT


def _preprocess(edge_index):
    """Self-loops, balanced node->tile binning, per-core index planes."""
    src = np.asarray(edge_index[0], dtype=np.int64)
    rel = np.asarray(edge_index[1], dtype=np.int64)
    dst = np.asarray(edge_index[2], dtype=np.int64)
    loop = np.arange(N, dtype=np.int64)
    src_f = np.concatenate([src, loop])
    dst_f = np.concatenate([dst, loop])
    rel_f = np.concatenate([rel, np.full(N, R, dtype=np.int64)])

    deg = np.bincount(dst_f, minlength=N)

    order = np.argsort(-deg, kind="stable")
    tile_of = np.empty(N, np.int64)
    slot_of = np.empty(N, np.int64)
    heap = [(0, t) for t in range(TILES)]
    heapq.heapify(heap)
    counts = np.zeros(TILES, np.int64)
    loads = np.zeros(TILES, np.int64)
    for n in order:
        while True:
            load, t = heapq.heappop(heap)
            if counts[t] < P:
                break
        tile_of[n] = t
        slot_of[n] = counts[t]
        counts[t] += 1
        loads[t] += deg[n]
        if counts[t] < P:
            heapq.heappush(heap, (loads[t], t))

    perm_pos = tile_of * P + slot_of          # node -> slot (tile-major)
    lut = _chunkrow_lut()

    bpt = max(1, int(-(-loads.max() // P)))   # blocks per tile (uniform)
    nblk = NT * bpt
    cap = bpt * P

    et = tile_of[dst_f]
    eorder = np.argsort(et, kind="stable")
    et_s = et[eorder]
    starts = np.searchsorted(et_s, np.arange(TILES))
    ends = np.searchsorted(et_s, np.arange(TILES), side="right")

    src_a = np.zeros((TILES, cap), np.int64)          # pad -> row 0
    rel_a = np.full((TILES, cap), R, np.int64)        # pad -> zero e-row
    seg_a = np.full((TILES, cap), 999, np.int64)      # pad -> no one-hot
    for t in range(TILES):
        idx = eorder[starts[t]:ends[t]]
        k = idx.shape[0]
        src_a[t, :k] = lut[perm_pos[src_f[idx]]]
        rel_a[t, :k] = rel_f[idx]
        seg_a[t, :k] = slot_of[dst_f[idx]]

    gsrc = np.zeros((W, P, nblk), np.int32)
    grel = np.zeros((W, P, nblk), np.int32)
    gseg = np.zeros((W, P, nblk), np.float32)
    for c in range(W):
        g0 = c * NT
        gsrc[c] = src_a[g0:g0 + NT].reshape(nblk, P).T
        grel[c] = rel_a[g0:g0 + NT].reshape(nblk, P).T
        gseg[c] = seg_a[g0:g0 + NT].reshape(nblk, P).T.astype(np.float32)

    return dict(bpt=bpt, nblk=nblk, perm_pos=perm_pos, lut=lut,
                gsrc=gsrc, grel=grel, gseg=gseg)


def _build(bpt, reps=1):
    nblk = NT * bpt
    nc = bacc.Bacc("TRN2", target_bir_lowering=False, debug=False, num_devices=W)

    # ---- per-core inputs
    gsrc = nc.declare_dram_parameter("gsrc", [P, nblk], I32, isOutput=False)
    gseg = nc.declare_dram_parameter("gseg", [P, nblk], F32, isOutput=False)
    gsegT = nc.declare_dram_parameter("gsegT", [nblk, P], BF16, isOutput=False)
    relE = nc.declare_dram_parameter("relE", [nblk * P, IN], BF16, isOutput=False)
    x_own = nc.declare_dram_parameter("x_own", [SHARD, IN], BF16, isOutput=False)
    # ---- replicated inputs
    x_slot = nc.declare_dram_parameter("x_slot", [NSLOT, IN], BF16, isOutput=False)
    wl1 = nc.declare_dram_parameter("wl1", [IN, HC1], F32, isOutput=False)
    wr1 = nc.declare_dram_parameter("wr1", [IN, HC1], F32, isOutput=False)
    we1 = nc.declare_dram_parameter("we1", [IN, HC1], F32, isOutput=False)
    att1f = nc.declare_dram_parameter("att1f", [1, HC1], F32, isOutput=False)
    eb1 = nc.declare_dram_parameter("eb1", [1, HC1], F32, isOutput=False)
    ob1 = nc.declare_dram_parameter("ob1", [1, HC1], F32, isOutput=False)
    wl2 = nc.declare_dram_parameter("wl2", [HC1, HC2], F32, isOutput=False)
    wr2 = nc.declare_dram_parameter("wr2", [HC1, HC2], F32, isOutput=False)
    we2 = nc.declare_dram_parameter("we2", [IN, HC2], F32, isOutput=False)
    att2f = nc.declare_dram_parameter("att2f", [1, HC2], F32, isOutput=False)
    eb2 = nc.declare_dram_parameter("eb2", [1, HC2], F32, isOutput=False)
    ob2 = nc.declare_dram_parameter("ob2", [1, OUT], F32, isOutput=False)
    out_p = nc.declare_dram_parameter("out", [SHARD, OUT], F32, isOutput=True)
    import os as _os
    _DBG = _os.environ.get("GAT_DEBUG", "0") == "1"
    if _DBG:
        dbgV = nc.declare_dram_parameter("dbgV", [P, HC1], BF16, isOutput=True)
        dbgMf = nc.declare_dram_parameter("dbgMf", [P, HC1], BF16, isOutput=True)
        dbglg = nc.declare_dram_parameter("dbglg", [P, 3 * H], F32, isOutput=True)
        dbgwf = nc.declare_dram_parameter("dbgwf", [P, HC1], BF16, isOutput=True)
        dbgRv = nc.declare_dram_parameter("dbgRv", [P, HC1], BF16, isOutput=True)
        dbgQ = nc.declare_dram_parameter("dbgQ", [P, P], BF16, isOutput=True)
        dbgh = nc.declare_dram_parameter("dbgh", [P, HC1], BF16, isOutput=True)
        dbgxr = nc.declare_dram_parameter("dbgxr", [P, HC1], BF16, isOutput=True)

    # ---- internal DRAM
    xl1_full = nc.dram_tensor("xl1_full", [NSLOT, HC1], BF16)
    h_shard = nc.dram_tensor("h_shard", [SHARD, HC1], BF16)
    xl2_shard = nc.dram_tensor("xl2_shard", [SHARD, HC2], BF16)
    xl2_full = nc.dram_tensor("xl2_full", [NSLOT, HC2], BF16, addr_space="Shared")

    RG = [list(range(W))]
    IOA = bass.IndirectOffsetOnAxis
    ACTF = mybir.ActivationFunctionType
    ALU = mybir.AluOpType
    K1 = 3 if bpt % 3 == 0 else 1       # superblock width, layer 1
    K2 = 3 if bpt % 3 == 0 else 2       # superblock width, layer 2

    def sb_splits(K):
        sp, j = [], 0
        while j < bpt:
            kk = min(K, bpt - j)
            sp.append((j, kk))
            j += kk
        return sp

    with tile.TileContext(nc) as tc:
        with tc.tile_pool(name="const", bufs=1) as cp:
            iotaF = cp.tile([P, P], F32, tag="iotaF")
            nc.gpsimd.iota(iotaF[:], pattern=[[1, P]], base=0,
                           channel_multiplier=0,
                           allow_small_or_imprecise_dtypes=True)
            iotaP = cp.tile([P, 1], F32, tag="iotaP")
            nc.gpsimd.iota(iotaP[:], pattern=[[1, 1]], base=0,
                           channel_multiplier=1,
                           allow_small_or_imprecise_dtypes=True)
            wl1b = cp.tile([IN, HC1], BF16, tag="wl1b")
            nc.gpsimd.dma_start(out=wl1b[:], in_=wl1[:])
            wr1b = cp.tile([IN, HC1], BF16, tag="wr1b")
            nc.gpsimd.dma_start(out=wr1b[:], in_=wr1[:])
            we1b = cp.tile([IN, HC1], BF16, tag="we1b")
            nc.gpsimd.dma_start(out=we1b[:], in_=we1[:])
            we2b = cp.tile([IN, HC2], BF16, tag="we2b")
            nc.gpsimd.dma_start(out=we2b[:], in_=we2[:])
            wl2b, wr2b = [], []
            for k in range(2):
                wl2bk = cp.tile([P, HC2], BF16, tag=f"wl2b{k}")
                nc.gpsimd.dma_start(out=wl2bk[:], in_=wl2[k * P:(k + 1) * P, :])
                wl2b.append(wl2bk)
                wr2bk = cp.tile([P, HC2], BF16, tag=f"wr2b{k}")
                nc.gpsimd.dma_start(out=wr2bk[:], in_=wr2[k * P:(k + 1) * P, :])
                wr2b.append(wr2bk)
            attB1 = cp.tile([P, K1 * HC1], BF16, tag="attB1")
            for j in range(K1):
                nc.gpsimd.dma_start(out=attB1[:, j * HC1:(j + 1) * HC1],
                                    in_=att1f[:].to_broadcast([P, HC1]))
            attB2 = cp.tile([P, K2 * HC2], BF16, tag="attB2")
            for j in range(K2):
                nc.gpsimd.dma_start(out=attB2[:, j * HC2:(j + 1) * HC2],
                                    in_=att2f[:].to_broadcast([P, HC2]))
            eb1B = cp.tile([P, HC1], F32, tag="eb1B")
            nc.sync.dma_start(out=eb1B[:], in_=eb1[:].to_broadcast([P, HC1]))
            ob1B = cp.tile([P, HC1], BF16, tag="ob1B")
            nc.gpsimd.dma_start(out=ob1B[:], in_=ob1[:].to_broadcast([P, HC1]))
            eb2B = cp.tile([P, HC2], F32, tag="eb2B")
            nc.sync.dma_start(out=eb2B[:], in_=eb2[:].to_broadcast([P, HC2]))
            ob2B = cp.tile([P, OUT], F32, tag="ob2B")
            nc.sync.dma_start(out=ob2B[:], in_=ob2[:].to_broadcast([P, OUT]))
            gsrc_t = cp.tile([P, nblk], I32, tag="gsrc_t")
            nc.sync.dma_start(out=gsrc_t[:], in_=gsrc[:])
            gseg_t = cp.tile([P, nblk], F32, tag="gseg_t")
            nc.sync.dma_start(out=gseg_t[:], in_=gseg[:])
            xr1_loc = cp.tile([P, NT * HC1], BF16, tag="xr1_loc")
            xr2_loc = cp.tile([P, NT * HC2], BF16, tag="xr2_loc")

            for _rep in range(reps):
                # ======== phase A: e-tables + xr1 + xl1_full ========
                with (
                    tc.tile_pool(name="pa_w", bufs=3) as wp,
                    tc.tile_pool(name="pa_p", bufs=2, space="PSUM") as pp,
                ):
                    # xr1 for the core's own shard (from x_own)
                    for t in range(NT):
                        xoT = wp.tile([P, P], BF16, tag="xoT")
                        nc.sync.dma_start(out=xoT[:],
                                          in_=x_own[t * P:(t + 1) * P, :],
                                          transpose=True)
                        psR = pp.tile([P, HC1], F32, tag="psR")
                        nc.tensor.matmul(psR[:], lhsT=xoT[:], rhs=wr1b[:],
                                         start=True, stop=True)
                        nc.vector.tensor_tensor(
                            out=xr1_loc[:, t * HC1:(t + 1) * HC1],
                            in0=psR[:], in1=eb1B[:], op=ALU.add)

                    # xl1_full for ALL tiles (identical on every core),
                    # two tiles per iteration
                    for g2 in range(0, TILES, 2):
                        psB = pp.tile([P, 2 * HC1], F32, tag="psB")
                        rows = []
                        for u in range(2):
                            g = g2 + u
                            row0 = _row0_of_tile(g)
                            rows.append(row0)
                            xT = wp.tile([P, P], BF16, tag=f"xT{u}")
                            nc.sync.dma_start(out=xT[:],
                                              in_=x_slot[row0:row0 + P, :],
                                              transpose=True)
                            nc.tensor.matmul(psB[:, u * HC1:(u + 1) * HC1],
                                             lhsT=xT[:], rhs=wl1b[:],
                                             start=True, stop=True)
                        xls = wp.tile([P, 2 * HC1], BF16, tag="xls")
                        if (g2 // 2) % 2 == 0:
                            nc.scalar.activation(xls[:], psB[:], ACTF.Copy)
                        else:
                            nc.vector.tensor_copy(xls[:], psB[:])
                        for u in range(2):
                            nc.sync.dma_start(
                                out=xl1_full[rows[u]:rows[u] + P, :],
                                in_=xls[:, u * HC1:(u + 1) * HC1])

                # ======== layer 1 edge loop (+ fused layer-2 transforms) ====
                with (
                    tc.tile_pool(name="l1_w", bufs=3) as wp,
                    tc.tile_pool(name="l1_p", bufs=2, space="PSUM") as pp,
                    tc.tile_pool(name="l1_a", bufs=2, space="PSUM") as pa,
                ):
                    for t in range(NT):
                        b0 = t * bpt
                        relT = wp.tile([P, bpt * P], BF16, tag="relT")
                        nc.sync.dma_start(
                            out=relT[:],
                            in_=relE[b0 * P:(b0 + bpt) * P, :],
                            transpose=True)

                        acc1t = pa.tile([P, HC1], F32, tag="acc1t", bufs=2)
                        accd1t = pa.tile([P, H], F32, tag="accd1t", bufs=2)
                        acc1 = acc1t[:]
                        accd1 = accd1t[:]
                        XR1 = xr1_loc[:, t * HC1:(t + 1) * HC1]

                        for (j0, kk) in sb_splits(K1):
                            sfx = "" if kk == K1 else f"_{kk}"
                            V1 = wp.tile([P, kk * HC1], BF16, tag=f"V1{sfx}")
                            for jj in range(kk):
                                b = b0 + j0 + jj
                                nc.gpsimd.indirect_dma_start(
                                    out=V1[:, jj * HC1:(jj + 1) * HC1],
                                    out_offset=None, in_=xl1_full[:],
                                    in_offset=IOA(ap=gsrc_t[:, b:b + 1],
                                                  axis=0))
                            Qt = wp.tile([P, kk * P], BF16, tag=f"Qt{sfx}")
                            for jj in range(kk):
                                nc.vector.tensor_scalar(
                                    out=Qt[:, jj * P:(jj + 1) * P],
                                    in0=iotaF[:],
                                    scalar1=gseg_t[:, b0 + j0 + jj:b0 + j0 + jj + 1],
                                    scalar2=None, op0=ALU.is_equal)
                            segB = wp.tile([P, kk * P], BF16, tag=f"segB{sfx}")
                            nc.scalar.dma_start(
                                out=segB[:],
                                in_=gsegT[b0 + j0:b0 + j0 + kk, :]
                                .rearrange("(o k) p -> o (k p)", o=1)
                                .to_broadcast([P, kk * P]))
                            Pbt = wp.tile([P, kk * P], BF16, tag=f"Pbt{sfx}")
                            nc.vector.tensor_scalar(
                                out=Pbt[:], in0=segB[:], scalar1=iotaP[:],
                                scalar2=None, op0=ALU.is_equal)
                            psM = pp.tile([P, kk * HC1], F32, tag=f"psM{sfx}", bufs=2)
                            for jj in range(kk):
                                j = j0 + jj
                                nc.tensor.matmul(
                                    psM[:, jj * HC1:(jj + 1) * HC1],
                                    lhsT=Pbt[:, jj * P:(jj + 1) * P],
                                    rhs=XR1, start=True, stop=False)
                                nc.tensor.matmul(
                                    psM[:, jj * HC1:(jj + 1) * HC1],
                                    lhsT=relT[:, j * P:(j + 1) * P],
                                    rhs=we1b[:], start=False, stop=True)
                            Mf = wp.tile([P, kk * HC1], BF16, tag=f"Mf{sfx}")
                            nc.vector.scalar_tensor_tensor(
                                out=Mf[:], in0=psM[:], scalar=1.0,
                                in1=V1[:], op0=ALU.mult, op1=ALU.add)
                            Mr = wp.tile([P, kk * HC1], BF16, tag=f"Mr{sfx}")
                            nc.scalar.activation(Mr[:], Mf[:], ACTF.Prelu,
                                                 alpha=0.2)
                            Tm = wp.tile([P, kk * HC1], BF16, tag=f"Tm{sfx}")
                            nc.vector.tensor_tensor(
                                out=Tm[:], in0=Mr[:],
                                in1=attB1[:, 0:kk * HC1], op=ALU.mult)
                            logit = wp.tile([P, kk * H], F32, tag=f"lg{sfx}")
                            nc.vector.tensor_reduce(
                                out=logit[:],
                                in_=Tm[:].rearrange("p (q c) -> p q c", c=HID),
                                axis=mybir.AxisListType.X, op=ALU.add)
                            wfb = wp.tile([P, kk * HC1], BF16, tag=f"wfb{sfx}")
                            nc.scalar.activation(
                                wfb[:].rearrange("p (q c) -> p q c", c=HID),
                                logit[:].rearrange("p (q o) -> p q o", o=1)
                                .to_broadcast([P, kk * H, HID]),
                                ACTF.Exp)
                            Rv = wp.tile([P, kk * HC1], BF16, tag=f"Rv{sfx}")
                            nc.gpsimd.tensor_tensor(
                                out=Rv[:], in0=wfb[:], in1=V1[:],
                                op=ALU.mult)
                            Wc = wp.tile([P, kk * H], BF16, tag=f"Wc{sfx}")
                            nc.vector.tensor_copy(
                                Wc[:],
                                wfb[:].rearrange("p (q c) -> p q c", c=HID)
                                [:, :, 0])
                            if _DBG and t == 0 and j0 == 0:
                                nc.sync.dma_start(out=dbgV[:], in_=V1[:, 0:HC1])
                                nc.sync.dma_start(out=dbgMf[:], in_=Mf[:, 0:HC1])
                                nc.sync.dma_start(out=dbglg[:], in_=logit[:, 0:3 * H])
                                nc.sync.dma_start(out=dbgwf[:], in_=wfb[:, 0:HC1])
                                nc.sync.dma_start(out=dbgRv[:], in_=Rv[:, 0:HC1])
                                nc.sync.dma_start(out=dbgQ[:], in_=Qt[:, 0:P])
                            for jj in range(kk):
                                j = j0 + jj
                                nc.tensor.matmul(
                                    acc1, lhsT=Qt[:, jj * P:(jj + 1) * P],
                                    rhs=Rv[:, jj * HC1:(jj + 1) * HC1],
                                    start=(j == 0), stop=(j == bpt - 1))
                                nc.tensor.matmul(
                                    accd1, lhsT=Qt[:, jj * P:(jj + 1) * P],
                                    rhs=Wc[:, jj * H:(jj + 1) * H],
                                    start=(j == 0), stop=(j == bpt - 1))

                        # epilogue: h, then xl2/xr2 transforms for this tile
                        dn1 = wp.tile([P, H], F32, tag="dn1")
                        nc.vector.tensor_scalar_add(dn1[:], accd1, 1e-20)
                        rec = wp.tile([P, H], F32, tag="rec")
                        nc.vector.reciprocal(rec[:], dn1[:])
                        htmp = wp.tile([P, HC1], BF16, tag="htmp")
                        for hh in range(H):
                            nc.scalar.activation(
                                htmp[:, hh * HID:(hh + 1) * HID],
                                acc1[:, hh * HID:(hh + 1) * HID],
                                ACTF.Copy, scale=rec[:, hh:hh + 1])
                        hsb = wp.tile([P, HC1], BF16, tag="hsb")
                        nc.vector.tensor_tensor(out=hsb[:], in0=htmp[:],
                                                in1=ob1B[:], op=ALU.add)
                        if _DBG and t == 0:
                            nc.sync.dma_start(out=dbgh[:], in_=hsb[:])
                            nc.sync.dma_start(
                                out=dbgxr[:], in_=xr1_loc[:, 0:HC1])
                        nc.sync.dma_start(out=h_shard[t * P:(t + 1) * P, :],
                                          in_=hsb[:])
                        hTt = wp.tile([P, HC1], BF16, tag="hTt")
                        for k in range(2):
                            nc.sync.dma_start(
                                out=hTt[:, k * P:(k + 1) * P],
                                in_=h_shard[t * P:(t + 1) * P,
                                            k * P:(k + 1) * P],
                                transpose=True)
                        psC2 = pp.tile([P, K1 * HC1], F32, tag="psM", bufs=2)
                        for k in range(2):
                            nc.tensor.matmul(psC2[:, 0:HC2],
                                             lhsT=hTt[:, k * P:(k + 1) * P],
                                             rhs=wl2b[k][:],
                                             start=(k == 0), stop=(k == 1))
                        xl2sb = wp.tile([P, HC2], BF16, tag="xl2sb")
                        nc.scalar.activation(xl2sb[:], psC2[:, 0:HC2], ACTF.Copy)
                        nc.sync.dma_start(out=xl2_shard[t * P:(t + 1) * P, :],
                                          in_=xl2sb[:])
                        psC3 = pp.tile([P, K1 * HC1], F32, tag="psM", bufs=2)
                        for k in range(2):
                            nc.tensor.matmul(psC3[:, 0:HC2],
                                             lhsT=hTt[:, k * P:(k + 1) * P],
                                             rhs=wr2b[k][:],
                                             start=(k == 0), stop=(k == 1))
                        nc.vector.tensor_tensor(
                            out=xr2_loc[:, t * HC2:(t + 1) * HC2],
                            in0=psC3[:, 0:HC2], in1=eb2B[:], op=ALU.add)

                        # chunked AllGather as soon as a chunk's tiles are done
                        for k in range(NCHUNK):
                            if t == CHUNK_T0[k] + CHUNK_TILES[k] - 1:
                                r0 = CHUNK_T0[k] * P
                                r1 = r0 + CHUNK_TILES[k] * P
                                o0 = CHUNK_BASE[k]
                                o1 = o0 + W * CHUNK_TILES[k] * P
                                nc.gpsimd.collective_compute(
                                    "AllGather", ALU.bypass,
                                    ins=[xl2_shard[r0:r1, :]],
                                    outs=[xl2_full[o0:o1, :]],
                                    replica_groups=RG)

                # ======== layer 2 edge loop ========
                with (
                    tc.tile_pool(name="l2_w", bufs=3) as wp,
                    tc.tile_pool(name="l2_p", bufs=2, space="PSUM") as pp,
                    tc.tile_pool(name="l2_a", bufs=1, space="PSUM") as pa,
                ):
                    for t in range(NT):
                        b0 = t * bpt
                        relT = wp.tile([P, bpt * P], BF16, tag="relT2")
                        nc.sync.dma_start(
                            out=relT[:],
                            in_=relE[b0 * P:(b0 + bpt) * P, :],
                            transpose=True)

                        acc2 = pa.tile([P, HC2], F32, tag="acc2", bufs=2)
                        accd2 = pa.tile([P, H], F32, tag="accd2", bufs=2)
                        XR2 = xr2_loc[:, t * HC2:(t + 1) * HC2]

                        for (j0, kk) in sb_splits(K2):
                            sfx = "" if kk == K2 else f"_{kk}"
                            V2 = wp.tile([P, kk * HC2], BF16, tag=f"V2{sfx}")
                            for jj in range(kk):
                                b = b0 + j0 + jj
                                nc.gpsimd.indirect_dma_start(
                                    out=V2[:, jj * HC2:(jj + 1) * HC2],
                                    out_offset=None, in_=xl2_full[:],
                                    in_offset=IOA(ap=gsrc_t[:, b:b + 1],
                                                  axis=0))
                            Qt = wp.tile([P, kk * P], BF16, tag=f"Qt{sfx}")
                            for jj in range(kk):
                                nc.vector.tensor_scalar(
                                    out=Qt[:, jj * P:(jj + 1) * P],
                                    in0=iotaF[:],
                                    scalar1=gseg_t[:, b0 + j0 + jj:b0 + j0 + jj + 1],
                                    scalar2=None, op0=ALU.is_equal)
                            segB = wp.tile([P, kk * P], BF16, tag=f"segB{sfx}")
                            nc.scalar.dma_start(
                                out=segB[:],
                                in_=gsegT[b0 + j0:b0 + j0 + kk, :]
                                .rearrange("(o k) p -> o (k p)", o=1)
                                .to_broadcast([P, kk * P]))
                            Pbt = wp.tile([P, kk * P], BF16, tag=f"Pbt{sfx}")
                            nc.vector.tensor_scalar(
                                out=Pbt[:], in0=segB[:], scalar1=iotaP[:],
                                scalar2=None, op0=ALU.is_equal)
                            psM = pp.tile([P, kk * HC2], F32, tag=f"psM{sfx}", bufs=1)
                            for jj in range(kk):
                                j = j0 + jj
                                nc.tensor.matmul(
                                    psM[:, jj * HC2:(jj + 1) * HC2],
                                    lhsT=Pbt[:, jj * P:(jj + 1) * P],
                                    rhs=XR2, start=True, stop=False)
                                nc.tensor.matmul(
                                    psM[:, jj * HC2:(jj + 1) * HC2],
                                    lhsT=relT[:, j * P:(j + 1) * P],
                                    rhs=we2b[:], start=False, stop=True)
                            Mf = wp.tile([P, kk * HC2], BF16, tag=f"Mf{sfx}")
                            nc.vector.scalar_tensor_tensor(
                                out=Mf[:], in0=psM[:], scalar=1.0,
                                in1=V2[:], op0=ALU.mult, op1=ALU.add)
                            Mr = wp.tile([P, kk * HC2], BF16, tag=f"Mr{sfx}")
                            nc.scalar.activation(Mr[:], Mf[:], ACTF.Prelu,
                                                 alpha=0.2)
                            Tm = wp.tile([P, kk * HC2], BF16, tag=f"Tm{sfx}")
                            nc.vector.tensor_tensor(
                                out=Tm[:], in0=Mr[:],
                                in1=attB2[:, 0:kk * HC2], op=ALU.mult)
                            logit = wp.tile([P, kk * H], F32, tag=f"lg{sfx}")
                            nc.vector.tensor_reduce(
                                out=logit[:],
                                in_=Tm[:].rearrange("p (q c) -> p q c", c=OUT),
                                axis=mybir.AxisListType.X, op=ALU.add)
                            wfb = wp.tile([P, kk * HC2], BF16, tag=f"wfb{sfx}")
                            nc.scalar.activation(
                                wfb[:].rearrange("p (q c) -> p q c", c=OUT),
                                logit[:].rearrange("p (q o) -> p q o", o=1)
                                .to_broadcast([P, kk * H, OUT]),
                                ACTF.Exp)
                            Rv = wp.tile([P, kk * HC2], BF16, tag=f"Rv{sfx}")
                            nc.gpsimd.tensor_tensor(
                                out=Rv[:], in0=wfb[:], in1=V2[:],
                                op=ALU.mult)
                            Wc = wp.tile([P, kk * H], BF16, tag=f"Wc{sfx}")
                            nc.vector.tensor_copy(
                                Wc[:],
                                wfb[:].rearrange("p (q c) -> p q c", c=OUT)
                                [:, :, 0])
                            for jj in range(kk):
                                j = j0 + jj
                                nc.tensor.matmul(
                                    acc2[:], lhsT=Qt[:, jj * P:(jj + 1) * P],
                                    rhs=Rv[:, jj * HC2:(jj + 1) * HC2],
                                    start=(j == 0), stop=(j == bpt - 1))
                                nc.tensor.matmul(
                                    accd2[:], lhsT=Qt[:, jj * P:(jj + 1) * P],
                                    rhs=Wc[:, jj * H:(jj + 1) * H],
                                    start=(j == 0), stop=(j == bpt - 1))

                        # epilogue: out = mean_h(acc_h/denom_h) + bias
                        dn2 = wp.tile([P, H], F32, tag="dn2")
                        nc.vector.tensor_scalar_add(dn2[:], accd2[:], 1e-20)
                        rec2 = wp.tile([P, H], F32, tag="rec2")
                        nc.vector.reciprocal(rec2[:], dn2[:])
                        rec4 = wp.tile([P, H], F32, tag="rec4")
                        nc.vector.tensor_scalar_mul(rec4[:], rec2[:], 0.25)
                        ho = wp.tile([P, H * OUT], F32, tag="ho")
                        for hh in range(H):
                            nc.scalar.activation(
                                ho[:, hh * OUT:(hh + 1) * OUT],
                                acc2[:, hh * OUT:(hh + 1) * OUT],
                                ACTF.Copy, scale=rec4[:, hh:hh + 1])
                        s01 = wp.tile([P, OUT], F32, tag="s01")
                        nc.vector.tensor_tensor(out=s01[:], in0=ho[:, 0:OUT],
                                                in1=ho[:, OUT:2 * OUT],
                                                op=ALU.add)
                        s23 = wp.tile([P, OUT], F32, tag="s23")
                        nc.gpsimd.tensor_tensor(out=s23[:],
                                                in0=ho[:, 2 * OUT:3 * OUT],
                                                in1=ho[:, 3 * OUT:4 * OUT],
                                                op=ALU.add)
                        s03 = wp.tile([P, OUT], F32, tag="s03")
                        nc.vector.tensor_tensor(out=s03[:], in0=s01[:],
                                                in1=s23[:], op=ALU.add)
                        osb = wp.tile([P, OUT], F32, tag="osb")
                        nc.gpsimd.tensor_tensor(out=osb[:], in0=s03[:],
                                                in1=ob2B[:], op=ALU.add)
                        nc.sync.dma_start(out=out_p[t * P:(t + 1) * P, :],
                                          in_=osb[:])

    nc.compile()
    return nc


def _make_in_maps(inp, pre):
    f32 = np.float32
    lut = pre["lut"]
    perm_pos = pre["perm_pos"]
    x = np.asarray(inp["x"], f32)
    x_cm = np.zeros((NSLOT, IN), BF)
    x_cm[lut[perm_pos]] = x.astype(BF)
    rel_padz = np.zeros((R + 1, IN), BF)
    rel_padz[:R] = np.asarray(inp["relations"], f32).astype(BF)
    a = lambda k: np.asarray(inp[k], f32)
    rep = dict(
        x_slot=x_cm,
        wl1=a("Wl1"), wr1=a("Wr1"), we1=a("We1"),
        att1f=a("att1").reshape(1, HC1),
        eb1=(a("bl1") + a("br1")).reshape(1, HC1),
        ob1=(a("bl1") + a("bias1")).reshape(1, HC1),
        wl2=a("Wl2"), wr2=a("Wr2"), we2=a("We2"),
        att2f=a("att2").reshape(1, HC2),
        eb2=(a("bl2") + a("br2")).reshape(1, HC2),
        ob2=(a("bl2").reshape(H, OUT).mean(axis=0) + a("bias2")).reshape(1, OUT),
    )
    # per-core own x rows (tile-major within the core)
    x_slot_tm = np.zeros((NSLOT, IN), BF)   # tile-major slot order
    x_slot_tm[perm_pos] = x.astype(BF)
    in_maps = []
    for c in range(W):
        m = dict(rep)
        m["gsrc"] = np.ascontiguousarray(pre["gsrc"][c])
        m["gseg"] = np.ascontiguousarray(pre["gseg"][c])
        m["gsegT"] = np.ascontiguousarray(pre["gseg"][c].T.astype(BF))
        m["relE"] = np.ascontiguousarray(
            rel_padz[np.minimum(pre["grel"][c].T.reshape(-1), R)])
        m["x_own"] = np.ascontiguousarray(
            x_slot_tm[c * SHARD:(c + 1) * SHARD])
        in_maps.append(m)
    return in_maps


_CACHE = {}


def kernel(x, edge_index, relations,
           Wl1, bl1, Wr1, br1, We1, att1, bias1,
           Wl2, bl2, Wr2, br2, We2, att2, bias2, **_unused):
    x = np.asarray(x, np.float32)
    edge_index = np.asarray(edge_index)
    relations = np.asarray(relations, np.float32)

    pre = _preprocess(edge_index)
    bpt = pre["bpt"]

    if bpt not in _CACHE:
        _CACHE[bpt] = _build(bpt)
    nc = _CACHE[bpt]

    in_maps = _make_in_maps(
        dict(x=x, relations=relations, Wl1=Wl1, bl1=bl1, Wr1=Wr1, br1=br1,
             We1=We1, att1=att1, bias1=bias1, Wl2=Wl2, bl2=bl2, Wr2=Wr2,
             br2=br2, We2=We2, att2=att2, bias2=bias2), pre)

    import os
    trace = os.environ.get("GAT_TRACE", "0") == "1"
    res = run_bass_kernel_spmd(nc, in_maps, list(range(W)), trace=trace)
    global LAST_EXEC_NS, LAST_RES
    LAST_EXEC_NS = res.exec_time_ns
    LAST_RES = res
    cat = np.concatenate([res.results[c]["out"] for c in range(W)], axis=0)
    return np.ascontiguousarray(cat[pre["perm_pos"]])


if __name__ == "__main__":
    pass
